# revision 5
# baseline (speedup 1.0000x reference)
"""Single-head causal attention (B=8, T=2048, D=1024, H=128) on 8 TRN2
NeuronCores — data-parallel over batch (one batch element per core).

Per-core dataflow (bf16 matmul compute, f32 accumulation):
  1. x [T, D] DMA'd naturally (scalar-engine HW queue), cast bf16 on DVE,
     transposed via DMA xbar transpose (sync-engine HW queue) into
     xT [d-part, d-tile, t].
  2. Projections: qT[h, T], kT[h, T] (N=512 chunks), v natural [t, h] per
     128-row tile with a ones column appended -> v_aug [t, 129].
  3. Scores TRANSPOSED: ST[k-tile 128, q 512] = kT_tile^T @ qT_chunk.
     exp(scale*ST) on ScalarE writes PT bf16 — already in the lhsT
     orientation PV needs (no P transposes). Causality: lower-left tiles
     skipped; diagonal tiles exp only the valid column range and zero the
     128x128 triangle via GpSimd affine_select on PT.
  4. O[q 128, 129] += PT_slice^T @ v_aug_tile accumulated over k tiles in
     PSUM; col 128 is the softmax denominator (ones column). Divide on DVE,
     DMA out (sync queue).
"""

import numpy as np

import concourse.bass as bass
import concourse.bacc as bacc
import concourse.mybir as mybir
import concourse.tile as tile
from concourse import bass_utils

B, T, D, H = 8, 2048, 1024, 128
P = 128
DT = D // P  # 8 d tiles
TT = T // P  # 16 t tiles
CH = 512  # q chunk width
QC = T // CH  # 4 q chunks
N_CORES = 8
SCALE = float(1.0 / np.sqrt(H))

F32 = mybir.dt.float32
BF16 = mybir.dt.bfloat16


def build_nc():
    nc = bacc.Bacc("TRN2", target_bir_lowering=False, debug=False)
    x = nc.dram_tensor("x", [T, D], F32, kind="ExternalInput").ap()
    wq_d = nc.dram_tensor("wq", [D, H], F32, kind="ExternalInput").ap()
    wk_d = nc.dram_tensor("wk", [D, H], F32, kind="ExternalInput").ap()
    wv_d = nc.dram_tensor("wv", [D, H], F32, kind="ExternalInput").ap()
    out = nc.dram_tensor("out", [T, H], F32, kind="ExternalOutput").ap()

    with tile.TileContext(nc) as tc:
        _build_body(nc, tc, x, wq_d, wk_d, wv_d, out)
    nc.compile()
    return nc


def _build_body(nc, tc, x, wq_d, wk_d, wv_d, out):
    with (
        tc.tile_pool(name="persist", bufs=1) as persist,
        tc.tile_pool(name="work", bufs=3) as work,
        tc.tile_pool(name="ps", bufs=1, space="PSUM") as ps,
    ):
        # ---- weights: [D, H] -> [p, dt, h] (scalar HW queue), cast bf16 ----
        w_bf = []
        for nm, wd in (("wq", wq_d), ("wk", wk_d), ("wv", wv_d)):
            wf = work.tile([P, DT, H], F32, tag="wf32", name=f"{nm}_f32")
            nc.scalar.dma_start(wf[:], wd.rearrange("(a p) h -> p a h", p=P))
            wb = persist.tile([P, DT, H], BF16, tag=f"{nm}_bf", name=f"{nm}_bf")
            nc.vector.tensor_copy(wb[:], wf[:])
            w_bf.append(wb)
        wq_bf, wk_bf, wv_bf = w_bf

        # ---- persistent activations ----
        xT = persist.tile([P, DT, T], BF16, tag="xT", name="xT")
        qT = persist.tile([P, T], BF16, tag="qT", name="qT")
        kT = persist.tile([P, T], BF16, tag="kT", name="kT")
        v_aug = persist.tile([P, TT, H + 1], BF16, tag="v_aug", name="v_aug")
        nc.gpsimd.memset(v_aug[:], 1.0)  # col H stays 1.0 (ones trick)

        # ---- phase 1: load x, cast, DMA-transpose, projections ----
        for c in range(QC):
            for tt in range(4 * c, 4 * c + 4):
                x_nat = work.tile([P, D], F32, tag="x_nat", name=f"x_nat{tt}")
                nc.scalar.dma_start(x_nat[:], x[tt * P : (tt + 1) * P, :])
                x_bf = work.tile([P, D], BF16, tag="x_bf", name=f"x_bf{tt}")
                nc.vector.tensor_copy(x_bf[:], x_nat[:])
                # xbar transpose: xT[p, a, tt*P + t] = x_bf[t, a*P + p]
                nc.sync.dma_start_transpose(
                    xT[:, :, tt * P : (tt + 1) * P], x_bf[:]
                )
            # qT / kT for this chunk of t
            for nm, wb, dstT in (("q", wq_bf, qT), ("k", wk_bf, kT)):
                pr_ps = ps.tile([P, CH], F32, tag="p1", bufs=2, name=f"{nm}T_ps{c}")
                for dt in range(DT):
                    nc.tensor.matmul(
                        pr_ps[:],
                        wb[:, dt, :],
                        xT[:, dt, c * CH : (c + 1) * CH],
                        start=(dt == 0),
                        stop=(dt == DT - 1),
                    )
                if nm == "q":
                    nc.vector.tensor_copy(dstT[:, c * CH : (c + 1) * CH], pr_ps[:])
                else:
                    nc.scalar.copy(dstT[:, c * CH : (c + 1) * CH], pr_ps[:])
            # v natural for the 4 t-tiles of this chunk
            for tt in range(4 * c, 4 * c + 4):
                v_ps = ps.tile([P, H], F32, tag="p1", bufs=2, name=f"v_ps{tt}")
                for dt in range(DT):
                    nc.tensor.matmul(
                        v_ps[:],
                        xT[:, dt, tt * P : (tt + 1) * P],
                        wv_bf[:, dt, :],
                        start=(dt == 0),
                        stop=(dt == DT - 1),
                    )
                nc.vector.tensor_copy(v_aug[:, tt, 0:H], v_ps[:])

        # ---- phase 2: attention main loop ----
        for c in range(QC):
            last = 4 * c + 3
            o_ps = [
                ps.tile([P, H + 1], F32, tag="o", bufs=4, name=f"o{c}_{s}")
                for s in range(4)
            ]
            st_ps = {}

            def emit_s(i, c=c, st_ps=st_ps):
                st = ps.tile([P, CH], F32, tag="st", bufs=2, name=f"st{c}_{i}")
                nc.tensor.matmul(
                    st[:],
                    kT[:, i * P : (i + 1) * P],
                    qT[:, c * CH : (c + 1) * CH],
                    start=True,
                    stop=True,
                )
                st_ps[i] = st

            emit_s(0)
            for i in range(last + 1):
                if i < last:
                    emit_s(i + 1)  # keep PE busy while ACT does exp(i)
                st = st_ps.pop(i)
                j = i - 4 * c  # >= 0 on the diagonal band
                e0 = max(j, 0) * P  # first valid q column in this chunk
                pt = work.tile([P, CH], BF16, tag="pt", name=f"pt{c}_{i}")
                nc.scalar.activation(
                    pt[:, e0:CH],
                    st[:, e0:CH],
                    mybir.ActivationFunctionType.Exp,
                    scale=SCALE,
                )
                if j >= 0:
                    # zero the causal triangle of the diagonal 128x128 block:
                    # keep where qq - kk >= 0
                    nc.gpsimd.affine_select(
                        out=pt[:, e0 : e0 + P],
                        in_=pt[:, e0 : e0 + P],
                        compare_op=mybir.AluOpType.is_ge,
                        fill=0.0,
                        base=0,
                        pattern=[[1, P]],
                        channel_multiplier=-1,
                    )
                for s in range(4):
                    if i <= 4 * c + s:
                        nc.tensor.matmul(
                            o_ps[s][:],
                            pt[:, s * P : (s + 1) * P],
                            v_aug[:, i, :],
                            start=(i == 0),
                            stop=(i == 4 * c + s),
                        )
            for s in range(4):
                qt_idx = 4 * c + s
                recip = work.tile([P, 1], F32, tag="recip", name=f"rcp{qt_idx}")
                nc.vector.reciprocal(recip[:], o_ps[s][:, H : H + 1])
                o_sb = work.tile([P, H], F32, tag="o_sb", name=f"o_sb{qt_idx}")
                nc.vector.tensor_scalar_mul(o_sb[:], o_ps[s][:, 0:H], recip[:])
                nc.sync.dma_start(out[qt_idx * P : (qt_idx + 1) * P, :], o_sb[:])


_NC_CACHE = None


def _get_nc():
    global _NC_CACHE
    if _NC_CACHE is None:
        _NC_CACHE = build_nc()
    return _NC_CACHE


def kernel(**inputs):
    x = np.ascontiguousarray(np.asarray(inputs["x"], dtype=np.float32))
    wq = np.ascontiguousarray(np.asarray(inputs["Wq"], dtype=np.float32))
    wk = np.ascontiguousarray(np.asarray(inputs["Wk"], dtype=np.float32))
    wv = np.ascontiguousarray(np.asarray(inputs["Wv"], dtype=np.float32))
    assert x.shape == (B, T, D)
    nc = _get_nc()
    in_maps = [
        {"x": np.ascontiguousarray(x[b]), "wq": wq, "wk": wk, "wv": wv}
        for b in range(N_CORES)
    ]
    res = bass_utils.run_bass_kernel_spmd(nc, in_maps, core_ids=list(range(N_CORES)))
    return np.stack([res.results[b]["out"] for b in range(N_CORES)], axis=0)


# revision 9
# speedup vs baseline: 1.5987x; 1.5987x over previous
"""Single-head causal attention (B=8, T=2048, D=1024, H=128) on 8 TRN2
NeuronCores — data-parallel over batch (one batch element per core).

Per-core dataflow (bf16 matmul compute, f32 accumulation):
  1. x [T, D] DMA'd naturally (alternating sync/scalar HW queues), cast
     bf16 on DVE, transposed on TensorE (128x128 tiles vs identity) into
     xT [d-part, d-tile, t]. (DMA xbar transpose measured ~25-50 GB/s —
     too slow for 4 MB, so the PE does it.)
  2. Projections: qT[h, T], kT[h, T] (N=512 chunks), v natural [t, h] per
     128-row tile with a ones column appended -> v_aug [t, 129].
  3. Scores TRANSPOSED: ST[k-tile 128, q 512] = kT_tile^T @ qT_chunk.
     exp(scale*ST) on ScalarE writes PT bf16 — already in the lhsT
     orientation PV needs (no P transposes). Causality: lower-left tiles
     skipped; diagonal tiles exp only the valid column range and zero the
     128x128 triangle via GpSimd affine_select on PT.
  4. O[q 128, 129] += PT_slice^T @ v_aug_tile accumulated over k tiles in
     PSUM; col 128 is the softmax denominator (ones column). Divide on DVE,
     DMA out (sync queue).
"""

import numpy as np

import concourse.bass as bass
import concourse.bacc as bacc
import concourse.mybir as mybir
import concourse.tile as tile
from concourse import bass_utils
from concourse.masks import make_identity

B, T, D, H = 8, 2048, 1024, 128
P = 128
DT = D // P  # 8 d tiles
TT = T // P  # 16 t tiles
CH = 512  # q chunk width
QC = T // CH  # 4 q chunks
N_CORES = 8
SCALE = float(1.0 / np.sqrt(H))

F32 = mybir.dt.float32
BF16 = mybir.dt.bfloat16


def build_nc():
    nc = bacc.Bacc("TRN2", target_bir_lowering=False, debug=False)
    x = nc.dram_tensor("x", [T, D], F32, kind="ExternalInput").ap()
    wq_d = nc.dram_tensor("wq", [D, H], F32, kind="ExternalInput").ap()
    wk_d = nc.dram_tensor("wk", [D, H], F32, kind="ExternalInput").ap()
    wv_d = nc.dram_tensor("wv", [D, H], F32, kind="ExternalInput").ap()
    out = nc.dram_tensor("out", [T, H], F32, kind="ExternalOutput").ap()

    with tile.TileContext(nc) as tc:
        _build_body(nc, tc, x, wq_d, wk_d, wv_d, out)
    nc.compile()
    return nc


def _build_body(nc, tc, x, wq_d, wk_d, wv_d, out):
    with (
        tc.tile_pool(name="persist", bufs=1) as persist,
        tc.tile_pool(name="work", bufs=3) as work,
        tc.tile_pool(name="ps", bufs=1, space="PSUM") as ps,
    ):
        # ---- weights: [D, H] -> [p, dt, h] (scalar HW queue), cast bf16 ----
        w_bf = []
        for nm, wd in (("wq", wq_d), ("wk", wk_d), ("wv", wv_d)):
            wf = work.tile([P, DT, H], F32, tag="wf32", name=f"{nm}_f32")
            nc.scalar.dma_start(wf[:], wd.rearrange("(a p) h -> p a h", p=P))
            wb = persist.tile([P, DT, H], BF16, tag=f"{nm}_bf", name=f"{nm}_bf")
            nc.vector.tensor_copy(wb[:], wf[:])
            w_bf.append(wb)
        wq_bf, wk_bf, wv_bf = w_bf

        # ---- persistent activations ----
        ident = persist.tile([P, P], BF16, tag="ident", name="ident")
        make_identity(nc, ident)
        xT = persist.tile([P, DT, T], BF16, tag="xT", name="xT")
        qT = persist.tile([P, T], BF16, tag="qT", name="qT")
        kT = persist.tile([P, T], BF16, tag="kT", name="kT")
        v_aug = persist.tile([P, TT, H + 1], BF16, tag="v_aug", name="v_aug")
        nc.gpsimd.memset(v_aug[:], 1.0)  # col H stays 1.0 (ones trick)

        # ---- phase 1: load x, cast, transpose on PE, projections ----
        for c in range(QC):
            for tt in range(4 * c, 4 * c + 4):
                x_nat = work.tile([P, D], F32, tag="x_nat", name=f"x_nat{tt}")
                ldeng = nc.sync if tt % 2 == 0 else nc.scalar
                ldeng.dma_start(x_nat[:], x[tt * P : (tt + 1) * P, :])
                x_bf = work.tile([P, D], BF16, tag="x_bf", name=f"x_bf{tt}")
                nc.vector.tensor_copy(x_bf[:], x_nat[:])
                for half in range(2):
                    tr_ps = ps.tile(
                        [P, 4 * P], BF16, tag="mm", bufs=2, name=f"tr{tt}_{half}"
                    )
                    for j in range(4):
                        dt = half * 4 + j
                        nc.tensor.transpose(
                            tr_ps[:, j * P : (j + 1) * P],
                            x_bf[:, dt * P : (dt + 1) * P],
                            ident,
                        )
                    dst = xT[:, half * 4 : half * 4 + 4, tt * P : (tt + 1) * P]
                    src = tr_ps.rearrange("p (a t) -> p a t", a=4)
                    if (tt + half) % 2 == 0:
                        nc.vector.tensor_copy(dst, src)
                    else:
                        nc.scalar.copy(dst, src)
            # qT / kT for this chunk of t
            for nm, wb, dstT in (("q", wq_bf, qT), ("k", wk_bf, kT)):
                pr_ps = ps.tile([P, CH], F32, tag="p1", bufs=2, name=f"{nm}T_ps{c}")
                for dt in range(DT):
                    nc.tensor.matmul(
                        pr_ps[:],
                        wb[:, dt, :],
                        xT[:, dt, c * CH : (c + 1) * CH],
                        start=(dt == 0),
                        stop=(dt == DT - 1),
                    )
                if nm == "q":
                    nc.vector.tensor_copy(dstT[:, c * CH : (c + 1) * CH], pr_ps[:])
                else:
                    nc.scalar.copy(dstT[:, c * CH : (c + 1) * CH], pr_ps[:])
            # v natural for the 4 t-tiles of this chunk
            for tt in range(4 * c, 4 * c + 4):
                v_ps = ps.tile([P, H], F32, tag="p1", bufs=2, name=f"v_ps{tt}")
                for dt in range(DT):
                    nc.tensor.matmul(
                        v_ps[:],
                        xT[:, dt, tt * P : (tt + 1) * P],
                        wv_bf[:, dt, :],
                        start=(dt == 0),
                        stop=(dt == DT - 1),
                    )
                nc.vector.tensor_copy(v_aug[:, tt, 0:H], v_ps[:])

        # ---- phase 2: attention main loop ----
        for c in range(QC):
            last = 4 * c + 3
            o_ps = [
                ps.tile([P, H + 1], F32, tag="o", bufs=4, name=f"o{c}_{s}")
                for s in range(4)
            ]
            st_ps = {}

            def emit_s(i, c=c, st_ps=st_ps):
                st = ps.tile([P, CH], F32, tag="mm", bufs=2, name=f"st{c}_{i}")
                nc.tensor.matmul(
                    st[:],
                    kT[:, i * P : (i + 1) * P],
                    qT[:, c * CH : (c + 1) * CH],
                    start=True,
                    stop=True,
                )
                st_ps[i] = st

            emit_s(0)
            for i in range(last + 1):
                if i < last:
                    emit_s(i + 1)  # keep PE busy while ACT does exp(i)
                st = st_ps.pop(i)
                j = i - 4 * c  # >= 0 on the diagonal band
                e0 = max(j, 0) * P  # first valid q column in this chunk
                pt = work.tile([P, CH], BF16, tag="pt", name=f"pt{c}_{i}")
                nc.scalar.activation(
                    pt[:, e0:CH],
                    st[:, e0:CH],
                    mybir.ActivationFunctionType.Exp,
                    scale=SCALE,
                )
                if j >= 0:
                    # zero the causal triangle of the diagonal 128x128 block:
                    # keep where qq - kk >= 0
                    nc.gpsimd.affine_select(
                        out=pt[:, e0 : e0 + P],
                        in_=pt[:, e0 : e0 + P],
                        compare_op=mybir.AluOpType.is_ge,
                        fill=0.0,
                        base=0,
                        pattern=[[1, P]],
                        channel_multiplier=-1,
                    )
                for s in range(4):
                    if i <= 4 * c + s:
                        nc.tensor.matmul(
                            o_ps[s][:],
                            pt[:, s * P : (s + 1) * P],
                            v_aug[:, i, :],
                            start=(i == 0),
                            stop=(i == 4 * c + s),
                        )
            for s in range(4):
                qt_idx = 4 * c + s
                recip = work.tile([P, 1], F32, tag="recip", name=f"rcp{qt_idx}")
                nc.vector.reciprocal(recip[:], o_ps[s][:, H : H + 1])
                o_sb = work.tile([P, H], F32, tag="o_sb", name=f"o_sb{qt_idx}")
                nc.vector.tensor_scalar_mul(o_sb[:], o_ps[s][:, 0:H], recip[:])
                nc.sync.dma_start(out[qt_idx * P : (qt_idx + 1) * P, :], o_sb[:])


_NC_CACHE = None


def _get_nc():
    global _NC_CACHE
    if _NC_CACHE is None:
        _NC_CACHE = build_nc()
    return _NC_CACHE


def kernel(**inputs):
    x = np.ascontiguousarray(np.asarray(inputs["x"], dtype=np.float32))
    wq = np.ascontiguousarray(np.asarray(inputs["Wq"], dtype=np.float32))
    wk = np.ascontiguousarray(np.asarray(inputs["Wk"], dtype=np.float32))
    wv = np.ascontiguousarray(np.asarray(inputs["Wv"], dtype=np.float32))
    assert x.shape == (B, T, D)
    nc = _get_nc()
    in_maps = [
        {"x": np.ascontiguousarray(x[b]), "wq": wq, "wk": wk, "wv": wv}
        for b in range(N_CORES)
    ]
    res = bass_utils.run_bass_kernel_spmd(nc, in_maps, core_ids=list(range(N_CORES)))
    return np.stack([res.results[b]["out"] for b in range(N_CORES)], axis=0)


# revision 12
# speedup vs baseline: 1.6092x; 1.0066x over previous
"""Single-head causal attention (B=8, T=2048, D=1024, H=128) on 8 TRN2
NeuronCores — data-parallel over batch (one batch element per core).

Per-core dataflow (bf16 matmul compute, f32 accumulation):
  1. x [T, D] DMA'd naturally (alternating sync/scalar HW queues), cast
     bf16 on DVE, transposed on TensorE (128x128 tiles vs identity) into
     xT [d-part, d-tile, t]. (DMA xbar transpose measured ~25-50 GB/s —
     too slow for 4 MB, so the PE does it.)
  2. Projections: qT[h, T], kT[h, T] (N=512 chunks), v natural [t, h] per
     128-row tile with a ones column appended -> v_aug [t, 129].
  3. Scores TRANSPOSED: ST[k-tile 128, q 512] = kT_tile^T @ qT_chunk.
     exp(scale*ST) on ScalarE writes PT bf16 — already in the lhsT
     orientation PV needs (no P transposes). Causality: lower-left tiles
     skipped; diagonal tiles exp only the valid column range and zero the
     128x128 triangle via GpSimd affine_select on PT.
  4. O[q 128, 129] += PT_slice^T @ v_aug_tile accumulated over k tiles in
     PSUM; col 128 is the softmax denominator (ones column). Divide on DVE,
     DMA out (sync queue).
"""

import numpy as np

import concourse.bass as bass
import concourse.bacc as bacc
import concourse.mybir as mybir
import concourse.tile as tile
from concourse import bass_utils
from concourse.masks import make_identity

B, T, D, H = 8, 2048, 1024, 128
P = 128
DT = D // P  # 8 d tiles
TT = T // P  # 16 t tiles
CH = 512  # q chunk width
QC = T // CH  # 4 q chunks
N_CORES = 8
SCALE = float(1.0 / np.sqrt(H))

F32 = mybir.dt.float32
BF16 = mybir.dt.bfloat16


def build_nc():
    nc = bacc.Bacc("TRN2", target_bir_lowering=False, debug=False)
    x = nc.dram_tensor("x", [T, D], F32, kind="ExternalInput").ap()
    wq_d = nc.dram_tensor("wq", [D, H], F32, kind="ExternalInput").ap()
    wk_d = nc.dram_tensor("wk", [D, H], F32, kind="ExternalInput").ap()
    wv_d = nc.dram_tensor("wv", [D, H], F32, kind="ExternalInput").ap()
    out = nc.dram_tensor("out", [T, H], F32, kind="ExternalOutput").ap()

    with tile.TileContext(nc) as tc:
        _build_body(nc, tc, x, wq_d, wk_d, wv_d, out)
    nc.compile()
    return nc


def _build_body(nc, tc, x, wq_d, wk_d, wv_d, out):
    with (
        tc.tile_pool(name="persist", bufs=1) as persist,
        tc.tile_pool(name="work", bufs=3) as work,
        tc.tile_pool(name="ps", bufs=1, space="PSUM") as ps,
    ):
        # ---- weights: [D, H] -> [p, dt, h] (scalar HW queue), cast bf16 ----
        w_bf = []
        for nm, wd in (("wq", wq_d), ("wk", wk_d), ("wv", wv_d)):
            wf = work.tile([P, DT, H], F32, tag="wf32", name=f"{nm}_f32")
            nc.scalar.dma_start(wf[:], wd.rearrange("(a p) h -> p a h", p=P))
            wb = persist.tile([P, DT, H], BF16, tag=f"{nm}_bf", name=f"{nm}_bf")
            nc.vector.tensor_copy(wb[:], wf[:])
            w_bf.append(wb)
        wq_bf, wk_bf, wv_bf = w_bf

        # ---- persistent activations ----
        ident = persist.tile([P, P], BF16, tag="ident", name="ident")
        make_identity(nc, ident)
        xT = persist.tile([P, DT, T], BF16, tag="xT", name="xT")
        qT = persist.tile([P, T], BF16, tag="qT", name="qT")
        kT = persist.tile([P, T], BF16, tag="kT", name="kT")
        v_aug = persist.tile([P, TT, H + 1], BF16, tag="v_aug", name="v_aug")
        nc.gpsimd.memset(v_aug[:], 1.0)  # col H stays 1.0 (ones trick)

        # ---- phase 1: load x, cast, transpose on PE, projections ----
        for c in range(QC):
            for tt in range(4 * c, 4 * c + 4):
                x_nat = work.tile([P, D], F32, tag="x_nat", bufs=4, name=f"x_nat{tt}")
                ldeng = nc.sync if tt % 2 == 0 else nc.scalar
                ldeng.dma_start(x_nat[:], x[tt * P : (tt + 1) * P, :])
                x_bf = work.tile([P, D], BF16, tag="x_bf", bufs=4, name=f"x_bf{tt}")
                nc.vector.tensor_copy(x_bf[:], x_nat[:])
                for half in range(2):
                    tr_ps = ps.tile(
                        [P, 4 * P], BF16, tag="mm", bufs=2, name=f"tr{tt}_{half}"
                    )
                    for j in range(4):
                        dt = half * 4 + j
                        nc.tensor.transpose(
                            tr_ps[:, j * P : (j + 1) * P],
                            x_bf[:, dt * P : (dt + 1) * P],
                            ident,
                        )
                    dst = xT[:, half * 4 : half * 4 + 4, tt * P : (tt + 1) * P]
                    src = tr_ps.rearrange("p (a t) -> p a t", a=4)
                    nc.scalar.copy(dst, src)
            # qT / kT for this chunk of t
            for nm, wb, dstT in (("q", wq_bf, qT), ("k", wk_bf, kT)):
                pr_ps = ps.tile([P, CH], F32, tag="p1", bufs=2, name=f"{nm}T_ps{c}")
                for dt in range(DT):
                    nc.tensor.matmul(
                        pr_ps[:],
                        wb[:, dt, :],
                        xT[:, dt, c * CH : (c + 1) * CH],
                        start=(dt == 0),
                        stop=(dt == DT - 1),
                    )
                nc.vector.tensor_copy(dstT[:, c * CH : (c + 1) * CH], pr_ps[:])
            # v natural for the 4 t-tiles of this chunk
            for tt in range(4 * c, 4 * c + 4):
                v_ps = ps.tile([P, H], F32, tag="p1", bufs=2, name=f"v_ps{tt}")
                for dt in range(DT):
                    nc.tensor.matmul(
                        v_ps[:],
                        xT[:, dt, tt * P : (tt + 1) * P],
                        wv_bf[:, dt, :],
                        start=(dt == 0),
                        stop=(dt == DT - 1),
                    )
                nc.vector.tensor_copy(v_aug[:, tt, 0:H], v_ps[:])

        # ---- phase 2: attention main loop ----
        for c in range(QC):
            last = 4 * c + 3
            o_ps = [
                ps.tile([P, H + 1], F32, tag="o", bufs=4, name=f"o{c}_{s}")
                for s in range(4)
            ]
            st_ps = {}

            def emit_s(i, c=c, st_ps=st_ps):
                st = ps.tile([P, CH], F32, tag="mm", bufs=2, name=f"st{c}_{i}")
                nc.tensor.matmul(
                    st[:],
                    kT[:, i * P : (i + 1) * P],
                    qT[:, c * CH : (c + 1) * CH],
                    start=True,
                    stop=True,
                )
                st_ps[i] = st

            emit_s(0)
            for i in range(last + 1):
                if i < last:
                    emit_s(i + 1)  # keep PE busy while ACT does exp(i)
                st = st_ps.pop(i)
                j = i - 4 * c  # >= 0 on the diagonal band
                e0 = max(j, 0) * P  # first valid q column in this chunk
                pt = work.tile([P, CH], BF16, tag="pt", name=f"pt{c}_{i}")
                nc.scalar.activation(
                    pt[:, e0:CH],
                    st[:, e0:CH],
                    mybir.ActivationFunctionType.Exp,
                    scale=SCALE,
                )
                if j >= 0:
                    # zero the causal triangle of the diagonal 128x128 block:
                    # keep where qq - kk >= 0
                    nc.gpsimd.affine_select(
                        out=pt[:, e0 : e0 + P],
                        in_=pt[:, e0 : e0 + P],
                        compare_op=mybir.AluOpType.is_ge,
                        fill=0.0,
                        base=0,
                        pattern=[[1, P]],
                        channel_multiplier=-1,
                    )
                for s in range(4):
                    if i <= 4 * c + s:
                        nc.tensor.matmul(
                            o_ps[s][:],
                            pt[:, s * P : (s + 1) * P],
                            v_aug[:, i, :],
                            start=(i == 0),
                            stop=(i == 4 * c + s),
                        )
            for s in range(4):
                qt_idx = 4 * c + s
                recip = work.tile([P, 1], F32, tag="recip", name=f"rcp{qt_idx}")
                nc.vector.reciprocal(recip[:], o_ps[s][:, H : H + 1])
                o_sb = work.tile([P, H], F32, tag="o_sb", name=f"o_sb{qt_idx}")
                nc.vector.tensor_scalar_mul(o_sb[:], o_ps[s][:, 0:H], recip[:])
                nc.sync.dma_start(out[qt_idx * P : (qt_idx + 1) * P, :], o_sb[:])


_NC_CACHE = None


def _get_nc():
    global _NC_CACHE
    if _NC_CACHE is None:
        _NC_CACHE = build_nc()
    return _NC_CACHE


def kernel(**inputs):
    x = np.ascontiguousarray(np.asarray(inputs["x"], dtype=np.float32))
    wq = np.ascontiguousarray(np.asarray(inputs["Wq"], dtype=np.float32))
    wk = np.ascontiguousarray(np.asarray(inputs["Wk"], dtype=np.float32))
    wv = np.ascontiguousarray(np.asarray(inputs["Wv"], dtype=np.float32))
    assert x.shape == (B, T, D)
    nc = _get_nc()
    in_maps = [
        {"x": np.ascontiguousarray(x[b]), "wq": wq, "wk": wk, "wv": wv}
        for b in range(N_CORES)
    ]
    res = bass_utils.run_bass_kernel_spmd(nc, in_maps, core_ids=list(range(N_CORES)))
    return np.stack([res.results[b]["out"] for b in range(N_CORES)], axis=0)
